# revision 12
# baseline (speedup 1.0000x reference)
"""AffineCoupling forward on 8 TRN2 NeuronCores (Bass/Tile).

Strategy
--------
Data-parallel: batch 16384 is split into 8 shards of 2048 rows; the MLP
weights (~84 MB) are replicated to every core. Inside a core the shard is
processed as 2 superblocks of 1024 rows.

Dataflow: activations live TRANSPOSED in SBUF ([channel-part, batch-free])
so all four layers of both MLPs chain on the TensorEngine without any
transposes; weights are the stationary operand in natural [K, N] layout.
Matmuls run in float32r (TF32-like, ~13-bit mantissa, full PE rate).

Host-side folding: ActNorm (y = x*exp(g)+c) is folded into the previous
layer's weights/bias; the output ActNorm's second half is folded into the
t-MLP's last layer and into the exp() bias of the coupling; the first half
is applied in natural layout with replicated per-channel vectors.
"""

import numpy as np
from contextlib import ExitStack

import concourse.bass as bass
import concourse.tile as tile
from concourse import bacc, mybir, bass_utils

F32 = mybir.dt.float32
F32R = mybir.dt.float32r

B = 16384
D = 1024
X1 = 512
HID = 2048
N_CORES = 8
BC = B // N_CORES          # rows per core = 2048
SB = 1024                  # superblock rows
NSB = BC // SB             # 2
NB = SB // 512             # 2 moving subtiles of 512 per superblock
BT = SB // 128             # 8 b-tiles of 128 per superblock
MC_H = HID // 128          # 16
KC_1 = X1 // 128           # 4
MC_4 = X1 // 128           # 4


def _fold_mlp(w1, b1, g1, c1, w2, b2, g2, c2, w3, b3, g3, c3, w4, b4):
    """Fold the three ActNorms into the linear layers. Returns fp32 arrays."""
    def fold(w, b, g, c):
        e = np.exp(g.astype(np.float32))
        return (w * e[None, :]).astype(np.float32), (b * e + c).astype(np.float32)

    w1f, b1f = fold(w1, b1, g1, c1)
    w2f, b2f = fold(w2, b2, g2, c2)
    w3f, b3f = fold(w3, b3, g3, c3)
    return w1f, b1f, w2f, b2f, w3f, b3f, w4.astype(np.float32), b4.astype(np.float32)


def _wblocks(w):
    """[K, N] -> [MC, 128(kp), KC, 128(mp)] so each DMA line is contiguous."""
    K, N = w.shape
    kc, mc = K // 128, N // 128
    return np.ascontiguousarray(
        w.reshape(kc, 128, mc, 128).transpose(2, 1, 0, 3)
    )


def _bblocks(b):
    """[N] -> [128, MC] (channel chunk on free dim, partition = channel%128)."""
    return np.ascontiguousarray(b.reshape(-1, 128).T)


def _build_program(sum_og: float):
    nc = bacc.Bacc("TRN2", target_bir_lowering=False, debug=False)

    x_d = nc.dram_tensor("x", [BC, D], F32, kind="ExternalInput").ap()
    out_d = nc.dram_tensor("out", [BC, D], F32, kind="ExternalOutput").ap()
    ld_d = nc.dram_tensor("ld", [BC // 512, 512], F32, kind="ExternalOutput").ap()

    wdr = {}
    for p in ("s", "t"):
        wdr[f"w1{p}"] = nc.dram_tensor(f"w1{p}", [MC_H, 128, KC_1, 128], F32R, kind="ExternalInput").ap()
        wdr[f"w2{p}"] = nc.dram_tensor(f"w2{p}", [MC_H, 128, MC_H, 128], F32R, kind="ExternalInput").ap()
        wdr[f"w3{p}"] = nc.dram_tensor(f"w3{p}", [MC_H, 128, MC_H, 128], F32R, kind="ExternalInput").ap()
        wdr[f"w4{p}"] = nc.dram_tensor(f"w4{p}", [MC_4, 128, MC_H, 128], F32R, kind="ExternalInput").ap()
        for i, mc in (("1", MC_H), ("2", MC_H), ("3", MC_H), ("4", MC_4)):
            wdr[f"b{i}{p}"] = nc.dram_tensor(f"b{i}{p}", [128, mc], F32, kind="ExternalInput").ap()

    og1e_d = nc.dram_tensor("og1e", [128, X1], F32, kind="ExternalInput").ap()
    oc1_d = nc.dram_tensor("oc1", [128, X1], F32, kind="ExternalInput").ap()
    og2_d = nc.dram_tensor("og2", [128, MC_4], F32, kind="ExternalInput").ap()
    ones_d = nc.dram_tensor("ones", [128, 1], F32R, kind="ExternalInput").ap()
    idr_d = nc.dram_tensor("idr", [128, 128], F32R, kind="ExternalInput").ap()
    idf_d = nc.dram_tensor("idf", [128, 128], F32, kind="ExternalInput").ap()

    with tile.TileContext(nc) as tc, ExitStack() as ctx:
        consts = ctx.enter_context(tc.tile_pool(name="consts", bufs=1))
        acts = ctx.enter_context(tc.tile_pool(name="acts", bufs=1))
        wpool = ctx.enter_context(tc.tile_pool(name="w", bufs=4))
        io_pool = ctx.enter_context(tc.tile_pool(name="io", bufs=2))
        mm_ps = ctx.enter_context(tc.tile_pool(name="mmps", bufs=4, space="PSUM"))
        tr_ps = ctx.enter_context(tc.tile_pool(name="trps", bufs=4, space="PSUM"))

        # --- constants ---
        wsb = {}
        for p in ("s", "t"):
            for i, mc in (("1", MC_H), ("2", MC_H), ("3", MC_H), ("4", MC_4)):
                t = consts.tile([128, mc], F32, tag=f"b{i}{p}")
                nc.gpsimd.dma_start(out=t, in_=wdr[f"b{i}{p}"])
                wsb[f"b{i}{p}"] = t
        og1e = consts.tile([128, X1], F32, tag="og1e")
        nc.gpsimd.dma_start(out=og1e, in_=og1e_d)
        oc1 = consts.tile([128, X1], F32, tag="oc1")
        nc.gpsimd.dma_start(out=oc1, in_=oc1_d)
        og2 = consts.tile([128, MC_4], F32, tag="og2")
        nc.gpsimd.dma_start(out=og2, in_=og2_d)
        ones = consts.tile([128, 1], F32R, tag="ones")
        nc.gpsimd.dma_start(out=ones, in_=ones_d)
        idf = consts.tile([128, 128], F32, tag="idf")
        nc.gpsimd.dma_start(out=idf, in_=idf_d)
        idr = consts.tile([128, 128], F32R, tag="idr")
        nc.gpsimd.dma_start(out=idr, in_=idr_d)

        # --- persistent activation buffers (shared across superblocks) ---
        x1T = acts.tile([128, KC_1, SB], F32R, tag="x1T")
        actA = acts.tile([128, MC_H, SB], F32R, tag="actA")
        actB = acts.tile([128, MC_H, SB], F32R, tag="actB")
        scaleT = acts.tile([128, MC_4, SB], F32R, tag="scaleT")
        trT = acts.tile([128, MC_4, SB], F32, tag="trT")

        def layer(wd, bias, src, kc_n, mc_n, drain, filler=None):
            """One linear layer: out[m-part, b-free] over kc_n k-chunks."""
            for m in range(mc_n):
                if filler is not None:
                    filler()
                ps = [mm_ps.tile([128, 512], F32, tag="mm", name=f"mm{b}")
                      for b in range(NB)]
                for kh in range(0, kc_n, 4):
                    kw = min(4, kc_n - kh)
                    wt = wpool.tile([128, kw, 128], F32R, tag="w")
                    nc.sync.dma_start(out=wt, in_=wd[m, :, kh:kh + kw, :])
                    for kk in range(kw):
                        k = kh + kk
                        for b in range(NB):
                            nc.tensor.matmul(
                                ps[b], wt[:, kk, :], src(k, b),
                                start=(k == 0), stop=(k == kc_n - 1),
                            )
                for b in range(NB):
                    drain(m, b, ps[b], bias)

        def relu_drain(dst):
            def f(m, b, ps, bias):
                nc.scalar.activation(
                    out=dst[:, m, b * 512:(b + 1) * 512], in_=ps,
                    func=mybir.ActivationFunctionType.Relu,
                    bias=bias[:, m:m + 1], scale=1.0,
                )
            return f

        def layer4(wd, bias, src, drain, mid_hook=None):
            # b-outer so the b=0 half of the output is complete early
            for b in range(NB):
                for m in range(MC_4):
                    ps = mm_ps.tile([128, 512], F32, tag="mm", name="mmL4")
                    for kh in range(0, MC_H, 4):
                        wt = wpool.tile([128, 4, 128], F32R, tag="w")
                        nc.sync.dma_start(out=wt, in_=wd[m, :, kh:kh + 4, :])
                        for kk in range(4):
                            k = kh + kk
                            nc.tensor.matmul(
                                ps, wt[:, kk, :], src(k, b),
                                start=(k == 0), stop=(k == MC_H - 1),
                            )
                    drain(m, b, ps, bias)
                if b == 0 and mid_hook is not None:
                    mid_hook()

        def mlp(p, last_drain, filler=None, post_l1=None, pre_l4=None,
                l4_mid=None):
            layer(wdr[f"w1{p}"], wsb[f"b1{p}"],
                  lambda k, b: x1T[:, k, b * 512:(b + 1) * 512],
                  KC_1, MC_H, relu_drain(actA), filler)
            if post_l1 is not None:
                post_l1()
            layer(wdr[f"w2{p}"], wsb[f"b2{p}"],
                  lambda k, b: actA[:, k, b * 512:(b + 1) * 512],
                  MC_H, MC_H, relu_drain(actB), filler)
            layer(wdr[f"w3{p}"], wsb[f"b3{p}"],
                  lambda k, b: actB[:, k, b * 512:(b + 1) * 512],
                  MC_H, MC_H, relu_drain(actA), filler)
            if pre_l4 is not None:
                pre_l4()
            layer4(wdr[f"w4{p}"], wsb[f"b4{p}"],
                  lambda k, b: actA[:, k, b * 512:(b + 1) * 512],
                  last_drain, l4_mid)

        def in_tr_bt(sb, bt):
            r0 = sb * SB
            xin = io_pool.tile([128, X1], F32R, tag="xin", bufs=3)
            nc.gpsimd.dma_start(
                out=xin, in_=x_d[r0 + bt * 128: r0 + (bt + 1) * 128, 0:X1].bitcast(F32R))
            for c in range(KC_1):
                tp = tr_ps.tile([128, 128], F32R, tag="trps")
                nc.tensor.transpose(tp, xin[:, c * 128:(c + 1) * 128], idr)
                nc.vector.tensor_copy(
                    out=x1T[:, c, bt * 128:(bt + 1) * 128], in_=tp)

        def in_transpose(sb):
            for bt in range(BT):
                in_tr_bt(sb, bt)

        def s_drain(m, b, ps, bias):
            nc.scalar.activation(
                out=scaleT[:, m, b * 512:(b + 1) * 512], in_=ps,
                func=mybir.ActivationFunctionType.Tanh,
                bias=bias[:, m:m + 1], scale=1.0,
            )

        def t_drain(m, b, ps, bias):
            nc.scalar.activation(
                out=trT[:, m, b * 512:(b + 1) * 512], in_=ps,
                func=mybir.ActivationFunctionType.Identity,
                bias=bias[:, m:m + 1], scale=1.0,
            )

        def ld_and_exp(sb):
            r0 = sb * SB
            # log_det = sum_c scale (channel sum via ones-matmul); +sum(og) on host
            for b in range(NB):
                lp = mm_ps.tile([1, 512], F32, tag="mm")
                for c in range(MC_4):
                    nc.tensor.matmul(
                        lp, ones, scaleT[:, c, b * 512:(b + 1) * 512],
                        start=(c == 0), stop=(c == MC_4 - 1),
                    )
                lrow = io_pool.tile([1, 512], F32, tag="ldrow", bufs=1)
                nc.scalar.copy(lrow, lp)
                nc.gpsimd.dma_start(
                    out=ld_d[(r0 // 512) + b: (r0 // 512) + b + 1, :], in_=lrow)
            # E = exp(scale + og2) in place (og2 folds output actnorm)
            for c in range(MC_4):
                nc.scalar.activation(
                    out=scaleT[:, c, :], in_=scaleT[:, c, :].bitcast(F32),
                    func=mybir.ActivationFunctionType.Exp,
                    bias=og2[:, c:c + 1], scale=1.0,
                )

        def couple_bt(sb, bt):
            r0 = sb * SB
            rows = slice(r0 + bt * 128, r0 + (bt + 1) * 128)
            onat = io_pool.tile([128, D], F32, tag="onat")
            nc.gpsimd.dma_start(out=onat[:, 0:X1], in_=x_d[rows, 0:X1])
            nc.gpsimd.dma_start(out=onat[:, X1:D], in_=x_d[rows, X1:D])

            # first half: x1 * exp(og1) + oc1 (per-channel on free dim)
            nc.vector.tensor_mul(onat[:, 0:X1], onat[:, 0:X1], og1e)
            nc.vector.tensor_add(onat[:, 0:X1], onat[:, 0:X1], oc1)

            # second half: x2 * E + T, transposing E and T back per block
            for c in range(MC_4):
                te = tr_ps.tile([128, 128], F32, tag="trps")
                nc.tensor.transpose(
                    te, scaleT[:, c, bt * 128:(bt + 1) * 128].bitcast(F32), idf)
                tt = tr_ps.tile([128, 128], F32, tag="trps")
                nc.tensor.transpose(
                    tt, trT[:, c, bt * 128:(bt + 1) * 128], idf)
                cs = slice(X1 + c * 128, X1 + (c + 1) * 128)
                nc.vector.tensor_mul(onat[:, cs], onat[:, cs], te)
                nc.vector.tensor_add(onat[:, cs], onat[:, cs], tt)

            nc.gpsimd.dma_start(out=out_d[rows, :], in_=onat)

        # Deferred-work queues: coupling of sb N-1 is spread through sb N's
        # s-MLP; input transposes of sb N+1 are spread through sb N's t-MLP
        # (emitted only after t-L1, which still reads sb N's x1T).
        pending = []
        fill_state = {"n": 0}

        def filler():
            fill_state["n"] += 1
            if pending and fill_state["n"] % 4 == 0:
                pending.pop(0)()

        def drain_pending():
            while pending:
                pending.pop(0)()

        for sb in range(NSB):
            if sb == 0:
                in_transpose(0)

            # s-MLP: fillers emit couple(sb-1); all must land before s-L4
            # (which overwrites scaleT that couple(sb-1) still reads).
            mlp("s", s_drain, filler, pre_l4=drain_pending)
            ld_and_exp(sb)

            def queue_next_in_tr(sb=sb):
                if sb + 1 < NSB:
                    pending.extend(
                        (lambda bt=bt: in_tr_bt(sb + 1, bt)) for bt in range(BT))

            last = sb == NSB - 1

            def couple_b0(sb=sb):
                for bt in range(BT // 2):
                    couple_bt(sb, bt)

            mlp("t", t_drain, filler, post_l1=queue_next_in_tr,
                pre_l4=drain_pending, l4_mid=couple_b0 if last else None)

            if last:
                for bt in range(BT // 2, BT):
                    couple_bt(sb, bt)
            else:
                pending.extend(
                    (lambda sb=sb, bt=bt: couple_bt(sb, bt)) for bt in range(BT))
        drain_pending()

    nc.compile()
    return nc


def _prep_inputs(inputs):
    """Host-side folding + layout. Returns the per-core common in_map pieces."""
    i = inputs
    og = np.asarray(i["og"], np.float32)
    oc = np.asarray(i["oc"], np.float32)
    og1e = np.exp(og[:X1])
    oc1 = oc[:X1]
    og2 = og[X1:]
    oc2 = oc[X1:]

    com = {}
    for p, pre in (("s", "s"), ("t", "t")):
        w1, b1, w2, b2, w3, b3, w4, b4 = _fold_mlp(
            *[np.asarray(i[f"{pre}{n}"]) for n in
              ("w1", "b1", "g1", "c1", "w2", "b2", "g2", "c2",
               "w3", "b3", "g3", "c3", "w4", "b4")])
        if p == "t":
            # fold second half of output actnorm into t-MLP's last layer
            e2 = np.exp(og2)
            w4 = (w4 * e2[None, :]).astype(np.float32)
            b4 = (b4 * e2 + oc2).astype(np.float32)
        com[f"w1{p}"] = _wblocks(w1)
        com[f"w2{p}"] = _wblocks(w2)
        com[f"w3{p}"] = _wblocks(w3)
        com[f"w4{p}"] = _wblocks(w4)
        com[f"b1{p}"] = _bblocks(b1)
        com[f"b2{p}"] = _bblocks(b2)
        com[f"b3{p}"] = _bblocks(b3)
        com[f"b4{p}"] = _bblocks(b4)

    com["og1e"] = np.ascontiguousarray(np.broadcast_to(og1e[None, :], (128, X1)))
    com["oc1"] = np.ascontiguousarray(np.broadcast_to(oc1[None, :], (128, X1)))
    com["og2"] = _bblocks(og2)
    com["ones"] = np.ones((128, 1), np.float32)
    com["idf"] = np.eye(128, dtype=np.float32)
    com["idr"] = np.eye(128, dtype=np.float32)
    sum_og = float(np.sum(og))
    return com, sum_og


def _run(inputs, trace=False, tmpdir=None):
    x = np.ascontiguousarray(np.asarray(inputs["x"], np.float32))
    assert x.shape == (B, D)
    com, sum_og = _prep_inputs(inputs)
    nc = _build_program(sum_og)

    in_maps = []
    for c in range(N_CORES):
        m = dict(com)
        m["x"] = np.ascontiguousarray(x[c * BC:(c + 1) * BC])
        in_maps.append(m)

    res = bass_utils.run_bass_kernel_spmd(
        nc, in_maps, core_ids=list(range(N_CORES)), trace=trace, tmpdir=tmpdir)

    out = np.concatenate([r["out"] for r in res.results], axis=0)
    ld = np.concatenate([r["ld"].reshape(-1) for r in res.results], axis=0) + np.float32(sum_og)
    return (out, ld), res


def kernel(**inputs):
    (out, ld), _ = _run(inputs, trace=False)
    return out, ld


# revision 13
# speedup vs baseline: 1.0498x; 1.0498x over previous
"""AffineCoupling forward on 8 TRN2 NeuronCores (Bass/Tile).

Strategy
--------
Data-parallel: batch 16384 is split into 8 shards of 2048 rows; the MLP
weights (~84 MB) are replicated to every core. Inside a core the shard is
processed as 2 superblocks of 1024 rows.

Dataflow: activations live TRANSPOSED in SBUF ([channel-part, batch-free])
so all four layers of both MLPs chain on the TensorEngine without any
transposes; weights are the stationary operand in natural [K, N] layout.
Matmuls run in float32r (TF32-like, ~13-bit mantissa, full PE rate).

Host-side folding: ActNorm (y = x*exp(g)+c) is folded into the previous
layer's weights/bias; the output ActNorm's second half is folded into the
t-MLP's last layer and into the exp() bias of the coupling; the first half
is applied in natural layout with replicated per-channel vectors.
"""

import numpy as np
from contextlib import ExitStack

import concourse.bass as bass
import concourse.tile as tile
from concourse import bacc, mybir, bass_utils

F32 = mybir.dt.float32
F32R = mybir.dt.float32r

B = 16384
D = 1024
X1 = 512
HID = 2048
N_CORES = 8
BC = B // N_CORES          # rows per core = 2048
SB = 1024                  # superblock rows
NSB = BC // SB             # 2
NB = SB // 512             # 2 moving subtiles of 512 per superblock
BT = SB // 128             # 8 b-tiles of 128 per superblock
MC_H = HID // 128          # 16
KC_1 = X1 // 128           # 4
MC_4 = X1 // 128           # 4


def _fold_mlp(w1, b1, g1, c1, w2, b2, g2, c2, w3, b3, g3, c3, w4, b4):
    """Fold the three ActNorms into the linear layers. Returns fp32 arrays."""
    def fold(w, b, g, c):
        e = np.exp(g.astype(np.float32))
        return (w * e[None, :]).astype(np.float32), (b * e + c).astype(np.float32)

    w1f, b1f = fold(w1, b1, g1, c1)
    w2f, b2f = fold(w2, b2, g2, c2)
    w3f, b3f = fold(w3, b3, g3, c3)
    return w1f, b1f, w2f, b2f, w3f, b3f, w4.astype(np.float32), b4.astype(np.float32)


def _wblocks(w):
    """[K, N] -> [MC, 128(kp), KC, 128(mp)] so each DMA line is contiguous."""
    K, N = w.shape
    kc, mc = K // 128, N // 128
    return np.ascontiguousarray(
        w.reshape(kc, 128, mc, 128).transpose(2, 1, 0, 3)
    )


def _bblocks(b):
    """[N] -> [128, MC] (channel chunk on free dim, partition = channel%128)."""
    return np.ascontiguousarray(b.reshape(-1, 128).T)


def _build_program(sum_og: float):
    nc = bacc.Bacc("TRN2", target_bir_lowering=False, debug=False)

    x_d = nc.dram_tensor("x", [BC, D], F32, kind="ExternalInput").ap()
    out_d = nc.dram_tensor("out", [BC, D], F32, kind="ExternalOutput").ap()
    ld_d = nc.dram_tensor("ld", [BC // 512, 512], F32, kind="ExternalOutput").ap()

    wdr = {}
    for p in ("s", "t"):
        wdr[f"w1{p}"] = nc.dram_tensor(f"w1{p}", [MC_H, 128, KC_1, 128], F32R, kind="ExternalInput").ap()
        wdr[f"w2{p}"] = nc.dram_tensor(f"w2{p}", [MC_H, 128, MC_H, 128], F32R, kind="ExternalInput").ap()
        wdr[f"w3{p}"] = nc.dram_tensor(f"w3{p}", [MC_H, 128, MC_H, 128], F32R, kind="ExternalInput").ap()
        wdr[f"w4{p}"] = nc.dram_tensor(f"w4{p}", [MC_4, 128, MC_H, 128], F32R, kind="ExternalInput").ap()
        for i, mc in (("1", MC_H), ("2", MC_H), ("3", MC_H), ("4", MC_4)):
            wdr[f"b{i}{p}"] = nc.dram_tensor(f"b{i}{p}", [128, mc], F32, kind="ExternalInput").ap()

    og1e_d = nc.dram_tensor("og1e", [128, X1], F32, kind="ExternalInput").ap()
    oc1_d = nc.dram_tensor("oc1", [128, X1], F32, kind="ExternalInput").ap()
    og2_d = nc.dram_tensor("og2", [128, MC_4], F32, kind="ExternalInput").ap()
    ones_d = nc.dram_tensor("ones", [128, 1], F32R, kind="ExternalInput").ap()
    idr_d = nc.dram_tensor("idr", [128, 128], F32R, kind="ExternalInput").ap()
    idf_d = nc.dram_tensor("idf", [128, 128], F32, kind="ExternalInput").ap()

    with tile.TileContext(nc) as tc, ExitStack() as ctx:
        consts = ctx.enter_context(tc.tile_pool(name="consts", bufs=1))
        acts = ctx.enter_context(tc.tile_pool(name="acts", bufs=1))
        wpool = ctx.enter_context(tc.tile_pool(name="w", bufs=4))
        io_pool = ctx.enter_context(tc.tile_pool(name="io", bufs=2))
        mm_ps = ctx.enter_context(tc.tile_pool(name="mmps", bufs=4, space="PSUM"))
        tr_ps = ctx.enter_context(tc.tile_pool(name="trps", bufs=4, space="PSUM"))

        # --- constants ---
        wsb = {}
        for p in ("s", "t"):
            for i, mc in (("1", MC_H), ("2", MC_H), ("3", MC_H), ("4", MC_4)):
                t = consts.tile([128, mc], F32, tag=f"b{i}{p}")
                nc.scalar.dma_start(out=t, in_=wdr[f"b{i}{p}"])
                wsb[f"b{i}{p}"] = t
        og1e = consts.tile([128, X1], F32, tag="og1e")
        nc.scalar.dma_start(out=og1e, in_=og1e_d)
        oc1 = consts.tile([128, X1], F32, tag="oc1")
        nc.scalar.dma_start(out=oc1, in_=oc1_d)
        og2 = consts.tile([128, MC_4], F32, tag="og2")
        nc.scalar.dma_start(out=og2, in_=og2_d)
        ones = consts.tile([128, 1], F32R, tag="ones")
        nc.scalar.dma_start(out=ones, in_=ones_d)
        idf = consts.tile([128, 128], F32, tag="idf")
        nc.scalar.dma_start(out=idf, in_=idf_d)
        idr = consts.tile([128, 128], F32R, tag="idr")
        nc.scalar.dma_start(out=idr, in_=idr_d)

        # --- persistent activation buffers (shared across superblocks) ---
        x1T = acts.tile([128, KC_1, SB], F32R, tag="x1T")
        actA = acts.tile([128, MC_H, SB], F32R, tag="actA")
        actB = acts.tile([128, MC_H, SB], F32R, tag="actB")
        scaleT = acts.tile([128, MC_4, SB], F32R, tag="scaleT")
        trT = acts.tile([128, MC_4, SB], F32, tag="trT")

        def layer(wd, bias, src, kc_n, mc_n, drain, filler=None):
            """One linear layer: out[m-part, b-free] over kc_n k-chunks."""
            for m in range(mc_n):
                if filler is not None:
                    filler()
                ps = [mm_ps.tile([128, 512], F32, tag="mm", name=f"mm{b}")
                      for b in range(NB)]
                for kh in range(0, kc_n, 4):
                    kw = min(4, kc_n - kh)
                    wt = wpool.tile([128, kw, 128], F32R, tag="w")
                    nc.sync.dma_start(out=wt, in_=wd[m, :, kh:kh + kw, :])
                    for kk in range(kw):
                        k = kh + kk
                        for b in range(NB):
                            nc.tensor.matmul(
                                ps[b], wt[:, kk, :], src(k, b),
                                start=(k == 0), stop=(k == kc_n - 1),
                            )
                for b in range(NB):
                    drain(m, b, ps[b], bias)

        def relu_drain(dst):
            def f(m, b, ps, bias):
                nc.scalar.activation(
                    out=dst[:, m, b * 512:(b + 1) * 512], in_=ps,
                    func=mybir.ActivationFunctionType.Relu,
                    bias=bias[:, m:m + 1], scale=1.0,
                )
            return f

        def layer4(wd, bias, src, drain, mid_hook=None):
            # b-outer so the b=0 half of the output is complete early
            for b in range(NB):
                for m in range(MC_4):
                    ps = mm_ps.tile([128, 512], F32, tag="mm", name="mmL4")
                    for kh in range(0, MC_H, 4):
                        wt = wpool.tile([128, 4, 128], F32R, tag="w")
                        nc.sync.dma_start(out=wt, in_=wd[m, :, kh:kh + 4, :])
                        for kk in range(4):
                            k = kh + kk
                            nc.tensor.matmul(
                                ps, wt[:, kk, :], src(k, b),
                                start=(k == 0), stop=(k == MC_H - 1),
                            )
                    drain(m, b, ps, bias)
                if b == 0 and mid_hook is not None:
                    mid_hook()

        def mlp(p, last_drain, filler=None, post_l1=None, pre_l4=None,
                l4_mid=None):
            layer(wdr[f"w1{p}"], wsb[f"b1{p}"],
                  lambda k, b: x1T[:, k, b * 512:(b + 1) * 512],
                  KC_1, MC_H, relu_drain(actA), filler)
            if post_l1 is not None:
                post_l1()
            layer(wdr[f"w2{p}"], wsb[f"b2{p}"],
                  lambda k, b: actA[:, k, b * 512:(b + 1) * 512],
                  MC_H, MC_H, relu_drain(actB), filler)
            layer(wdr[f"w3{p}"], wsb[f"b3{p}"],
                  lambda k, b: actB[:, k, b * 512:(b + 1) * 512],
                  MC_H, MC_H, relu_drain(actA), filler)
            if pre_l4 is not None:
                pre_l4()
            layer4(wdr[f"w4{p}"], wsb[f"b4{p}"],
                  lambda k, b: actA[:, k, b * 512:(b + 1) * 512],
                  last_drain, l4_mid)

        def in_tr_bt(sb, bt):
            r0 = sb * SB
            xin = io_pool.tile([128, X1], F32R, tag="xin", bufs=3)
            nc.scalar.dma_start(
                out=xin, in_=x_d[r0 + bt * 128: r0 + (bt + 1) * 128, 0:X1].bitcast(F32R))
            for c in range(KC_1):
                tp = tr_ps.tile([128, 128], F32R, tag="trps")
                nc.tensor.transpose(tp, xin[:, c * 128:(c + 1) * 128], idr)
                nc.vector.tensor_copy(
                    out=x1T[:, c, bt * 128:(bt + 1) * 128], in_=tp)

        def in_transpose(sb):
            for bt in range(BT):
                in_tr_bt(sb, bt)

        def s_drain(m, b, ps, bias):
            nc.scalar.activation(
                out=scaleT[:, m, b * 512:(b + 1) * 512], in_=ps,
                func=mybir.ActivationFunctionType.Tanh,
                bias=bias[:, m:m + 1], scale=1.0,
            )

        def t_drain(m, b, ps, bias):
            nc.scalar.activation(
                out=trT[:, m, b * 512:(b + 1) * 512], in_=ps,
                func=mybir.ActivationFunctionType.Identity,
                bias=bias[:, m:m + 1], scale=1.0,
            )

        def ld_and_exp(sb):
            r0 = sb * SB
            # log_det = sum_c scale (channel sum via ones-matmul); +sum(og) on host
            for b in range(NB):
                lp = mm_ps.tile([1, 512], F32, tag="mm")
                for c in range(MC_4):
                    nc.tensor.matmul(
                        lp, ones, scaleT[:, c, b * 512:(b + 1) * 512],
                        start=(c == 0), stop=(c == MC_4 - 1),
                    )
                lrow = io_pool.tile([1, 512], F32, tag="ldrow", bufs=1)
                nc.scalar.copy(lrow, lp)
                nc.scalar.dma_start(
                    out=ld_d[(r0 // 512) + b: (r0 // 512) + b + 1, :], in_=lrow)
            # E = exp(scale + og2) in place (og2 folds output actnorm)
            for c in range(MC_4):
                nc.scalar.activation(
                    out=scaleT[:, c, :], in_=scaleT[:, c, :].bitcast(F32),
                    func=mybir.ActivationFunctionType.Exp,
                    bias=og2[:, c:c + 1], scale=1.0,
                )

        def couple_bt(sb, bt):
            r0 = sb * SB
            rows = slice(r0 + bt * 128, r0 + (bt + 1) * 128)
            onat = io_pool.tile([128, D], F32, tag="onat")
            nc.scalar.dma_start(out=onat[:, 0:X1], in_=x_d[rows, 0:X1])
            nc.scalar.dma_start(out=onat[:, X1:D], in_=x_d[rows, X1:D])

            # first half: x1 * exp(og1) + oc1 (per-channel on free dim)
            nc.vector.tensor_mul(onat[:, 0:X1], onat[:, 0:X1], og1e)
            nc.vector.tensor_add(onat[:, 0:X1], onat[:, 0:X1], oc1)

            # second half: x2 * E + T, transposing E and T back per block
            for c in range(MC_4):
                te = tr_ps.tile([128, 128], F32, tag="trps")
                nc.tensor.transpose(
                    te, scaleT[:, c, bt * 128:(bt + 1) * 128].bitcast(F32), idf)
                tt = tr_ps.tile([128, 128], F32, tag="trps")
                nc.tensor.transpose(
                    tt, trT[:, c, bt * 128:(bt + 1) * 128], idf)
                cs = slice(X1 + c * 128, X1 + (c + 1) * 128)
                nc.vector.tensor_mul(onat[:, cs], onat[:, cs], te)
                nc.vector.tensor_add(onat[:, cs], onat[:, cs], tt)

            nc.scalar.dma_start(out=out_d[rows, :], in_=onat)

        # Deferred-work queues: coupling of sb N-1 is spread through sb N's
        # s-MLP; input transposes of sb N+1 are spread through sb N's t-MLP
        # (emitted only after t-L1, which still reads sb N's x1T).
        pending = []
        fill_state = {"n": 0}

        def filler():
            fill_state["n"] += 1
            if pending and fill_state["n"] % 4 == 0:
                pending.pop(0)()

        def drain_pending():
            while pending:
                pending.pop(0)()

        for sb in range(NSB):
            if sb == 0:
                in_transpose(0)

            # s-MLP: fillers emit couple(sb-1); all must land before s-L4
            # (which overwrites scaleT that couple(sb-1) still reads).
            mlp("s", s_drain, filler, pre_l4=drain_pending)
            ld_and_exp(sb)

            def queue_next_in_tr(sb=sb):
                if sb + 1 < NSB:
                    pending.extend(
                        (lambda bt=bt: in_tr_bt(sb + 1, bt)) for bt in range(BT))

            last = sb == NSB - 1

            def couple_b0(sb=sb):
                for bt in range(BT // 2):
                    couple_bt(sb, bt)

            mlp("t", t_drain, filler, post_l1=queue_next_in_tr,
                pre_l4=drain_pending, l4_mid=couple_b0 if last else None)

            if last:
                for bt in range(BT // 2, BT):
                    couple_bt(sb, bt)
            else:
                pending.extend(
                    (lambda sb=sb, bt=bt: couple_bt(sb, bt)) for bt in range(BT))
        drain_pending()

    nc.compile()
    return nc


def _prep_inputs(inputs):
    """Host-side folding + layout. Returns the per-core common in_map pieces."""
    i = inputs
    og = np.asarray(i["og"], np.float32)
    oc = np.asarray(i["oc"], np.float32)
    og1e = np.exp(og[:X1])
    oc1 = oc[:X1]
    og2 = og[X1:]
    oc2 = oc[X1:]

    com = {}
    for p, pre in (("s", "s"), ("t", "t")):
        w1, b1, w2, b2, w3, b3, w4, b4 = _fold_mlp(
            *[np.asarray(i[f"{pre}{n}"]) for n in
              ("w1", "b1", "g1", "c1", "w2", "b2", "g2", "c2",
               "w3", "b3", "g3", "c3", "w4", "b4")])
        if p == "t":
            # fold second half of output actnorm into t-MLP's last layer
            e2 = np.exp(og2)
            w4 = (w4 * e2[None, :]).astype(np.float32)
            b4 = (b4 * e2 + oc2).astype(np.float32)
        com[f"w1{p}"] = _wblocks(w1)
        com[f"w2{p}"] = _wblocks(w2)
        com[f"w3{p}"] = _wblocks(w3)
        com[f"w4{p}"] = _wblocks(w4)
        com[f"b1{p}"] = _bblocks(b1)
        com[f"b2{p}"] = _bblocks(b2)
        com[f"b3{p}"] = _bblocks(b3)
        com[f"b4{p}"] = _bblocks(b4)

    com["og1e"] = np.ascontiguousarray(np.broadcast_to(og1e[None, :], (128, X1)))
    com["oc1"] = np.ascontiguousarray(np.broadcast_to(oc1[None, :], (128, X1)))
    com["og2"] = _bblocks(og2)
    com["ones"] = np.ones((128, 1), np.float32)
    com["idf"] = np.eye(128, dtype=np.float32)
    com["idr"] = np.eye(128, dtype=np.float32)
    sum_og = float(np.sum(og))
    return com, sum_og


def _run(inputs, trace=False, tmpdir=None):
    x = np.ascontiguousarray(np.asarray(inputs["x"], np.float32))
    assert x.shape == (B, D)
    com, sum_og = _prep_inputs(inputs)
    nc = _build_program(sum_og)

    in_maps = []
    for c in range(N_CORES):
        m = dict(com)
        m["x"] = np.ascontiguousarray(x[c * BC:(c + 1) * BC])
        in_maps.append(m)

    res = bass_utils.run_bass_kernel_spmd(
        nc, in_maps, core_ids=list(range(N_CORES)), trace=trace, tmpdir=tmpdir)

    out = np.concatenate([r["out"] for r in res.results], axis=0)
    ld = np.concatenate([r["ld"].reshape(-1) for r in res.results], axis=0) + np.float32(sum_og)
    return (out, ld), res


def kernel(**inputs):
    (out, ld), _ = _run(inputs, trace=False)
    return out, ld


# revision 15
# speedup vs baseline: 1.0579x; 1.0077x over previous
"""AffineCoupling forward on 8 TRN2 NeuronCores (Bass/Tile).

Strategy
--------
Data-parallel: batch 16384 is split into 8 shards of 2048 rows; the MLP
weights (~84 MB) are replicated to every core. Inside a core the shard is
processed as 2 superblocks of 1024 rows.

Dataflow: activations live TRANSPOSED in SBUF ([channel-part, batch-free])
so all four layers of both MLPs chain on the TensorEngine without any
transposes; weights are the stationary operand in natural [K, N] layout.
Matmuls run in float32r (TF32-like, ~13-bit mantissa, full PE rate).

Host-side folding: ActNorm (y = x*exp(g)+c) is folded into the previous
layer's weights/bias; the output ActNorm's second half is folded into the
t-MLP's last layer and into the exp() bias of the coupling; the first half
is applied in natural layout with replicated per-channel vectors.
"""

import numpy as np
from contextlib import ExitStack

import concourse.bass as bass
import concourse.tile as tile
from concourse import bacc, mybir, bass_utils

F32 = mybir.dt.float32
F32R = mybir.dt.float32r

B = 16384
D = 1024
X1 = 512
HID = 2048
N_CORES = 8
BC = B // N_CORES          # rows per core = 2048
SB = 1024                  # superblock rows
NSB = BC // SB             # 2
NB = SB // 512             # 2 moving subtiles of 512 per superblock
BT = SB // 128             # 8 b-tiles of 128 per superblock
MC_H = HID // 128          # 16
KC_1 = X1 // 128           # 4
MC_4 = X1 // 128           # 4


def _fold_mlp(w1, b1, g1, c1, w2, b2, g2, c2, w3, b3, g3, c3, w4, b4):
    """Fold the three ActNorms into the linear layers. Returns fp32 arrays."""
    def fold(w, b, g, c):
        e = np.exp(g.astype(np.float32))
        return (w * e[None, :]).astype(np.float32), (b * e + c).astype(np.float32)

    w1f, b1f = fold(w1, b1, g1, c1)
    w2f, b2f = fold(w2, b2, g2, c2)
    w3f, b3f = fold(w3, b3, g3, c3)
    return w1f, b1f, w2f, b2f, w3f, b3f, w4.astype(np.float32), b4.astype(np.float32)


def _wblocks(w):
    """[K, N] -> [MC, 128(kp), KC, 128(mp)] so each DMA line is contiguous."""
    K, N = w.shape
    kc, mc = K // 128, N // 128
    return np.ascontiguousarray(
        w.reshape(kc, 128, mc, 128).transpose(2, 1, 0, 3)
    )


def _bblocks(b):
    """[N] -> [128, MC] (channel chunk on free dim, partition = channel%128)."""
    return np.ascontiguousarray(b.reshape(-1, 128).T)


def _build_program(sum_og: float):
    nc = bacc.Bacc("TRN2", target_bir_lowering=False, debug=False)

    x_d = nc.dram_tensor("x", [BC, D], F32, kind="ExternalInput").ap()
    out_d = nc.dram_tensor("out", [BC, D], F32, kind="ExternalOutput").ap()
    ld_d = nc.dram_tensor("ld", [BC // 512, 512], F32, kind="ExternalOutput").ap()

    wdr = {}
    for p in ("s", "t"):
        wdr[f"w1{p}"] = nc.dram_tensor(f"w1{p}", [MC_H, 128, KC_1, 128], F32R, kind="ExternalInput").ap()
        wdr[f"w2{p}"] = nc.dram_tensor(f"w2{p}", [MC_H, 128, MC_H, 128], F32R, kind="ExternalInput").ap()
        wdr[f"w3{p}"] = nc.dram_tensor(f"w3{p}", [MC_H, 128, MC_H, 128], F32R, kind="ExternalInput").ap()
        wdr[f"w4{p}"] = nc.dram_tensor(f"w4{p}", [MC_4, 128, MC_H, 128], F32R, kind="ExternalInput").ap()
        for i, mc in (("1", MC_H), ("2", MC_H), ("3", MC_H), ("4", MC_4)):
            wdr[f"b{i}{p}"] = nc.dram_tensor(f"b{i}{p}", [128, mc], F32, kind="ExternalInput").ap()

    og1e_d = nc.dram_tensor("og1e", [128, X1], F32, kind="ExternalInput").ap()
    oc1_d = nc.dram_tensor("oc1", [128, X1], F32, kind="ExternalInput").ap()
    og2_d = nc.dram_tensor("og2", [128, MC_4], F32, kind="ExternalInput").ap()
    ones_d = nc.dram_tensor("ones", [128, 1], F32R, kind="ExternalInput").ap()
    idr_d = nc.dram_tensor("idr", [128, 128], F32R, kind="ExternalInput").ap()
    idf_d = nc.dram_tensor("idf", [128, 128], F32, kind="ExternalInput").ap()

    with tile.TileContext(nc) as tc, ExitStack() as ctx:
        consts = ctx.enter_context(tc.tile_pool(name="consts", bufs=1))
        acts = ctx.enter_context(tc.tile_pool(name="acts", bufs=1))
        wpool = ctx.enter_context(tc.tile_pool(name="w", bufs=4))
        io_pool = ctx.enter_context(tc.tile_pool(name="io", bufs=2))
        mm_ps = ctx.enter_context(tc.tile_pool(name="mmps", bufs=4, space="PSUM"))
        tr_ps = ctx.enter_context(tc.tile_pool(name="trps", bufs=4, space="PSUM"))

        # --- identities + PE warmup first (HAM warm before L1; xin DMAs early) ---
        idf = consts.tile([128, 128], F32, tag="idf")
        nc.scalar.dma_start(out=idf, in_=idf_d)
        idr = consts.tile([128, 128], F32R, tag="idr")
        nc.scalar.dma_start(out=idr, in_=idr_d)
        warm_ps = mm_ps.tile([128, 128], F32, tag="mm", name="warmps")
        for wi in range(40):
            nc.tensor.matmul(warm_ps, idr, idr,
                             start=(wi == 0), stop=(wi == 39))
        warm_sb = consts.tile([128, 128], F32, tag="warmsb")
        nc.vector.tensor_copy(out=warm_sb, in_=warm_ps)

        # --- constants ---
        wsb = {}
        for p in ("s", "t"):
            for i, mc in (("1", MC_H), ("2", MC_H), ("3", MC_H), ("4", MC_4)):
                t = consts.tile([128, mc], F32, tag=f"b{i}{p}")
                nc.scalar.dma_start(out=t, in_=wdr[f"b{i}{p}"])
                wsb[f"b{i}{p}"] = t
        og1e = consts.tile([128, X1], F32, tag="og1e")
        nc.scalar.dma_start(out=og1e, in_=og1e_d)
        oc1 = consts.tile([128, X1], F32, tag="oc1")
        nc.scalar.dma_start(out=oc1, in_=oc1_d)
        og2 = consts.tile([128, MC_4], F32, tag="og2")
        nc.scalar.dma_start(out=og2, in_=og2_d)
        ones = consts.tile([128, 1], F32R, tag="ones")
        nc.scalar.dma_start(out=ones, in_=ones_d)

        # --- persistent activation buffers (shared across superblocks) ---
        x1T = acts.tile([128, KC_1, SB], F32R, tag="x1T")
        actA = acts.tile([128, MC_H, SB], F32R, tag="actA")
        actB = acts.tile([128, MC_H, SB], F32R, tag="actB")
        scaleT = acts.tile([128, MC_4, SB], F32R, tag="scaleT")
        trT = acts.tile([128, MC_4, SB], F32, tag="trT")

        def layer(wd, bias, src, kc_n, mc_n, drain, filler=None):
            """One linear layer: out[m-part, b-free] over kc_n k-chunks."""
            for m in range(mc_n):
                if filler is not None:
                    filler()
                ps = [mm_ps.tile([128, 512], F32, tag="mm", name=f"mm{b}")
                      for b in range(NB)]
                for kh in range(0, kc_n, 4):
                    kw = min(4, kc_n - kh)
                    wt = wpool.tile([128, kw, 128], F32R, tag="w")
                    nc.sync.dma_start(out=wt, in_=wd[m, :, kh:kh + kw, :])
                    for kk in range(kw):
                        k = kh + kk
                        for b in range(NB):
                            nc.tensor.matmul(
                                ps[b], wt[:, kk, :], src(k, b),
                                start=(k == 0), stop=(k == kc_n - 1),
                            )
                for b in range(NB):
                    drain(m, b, ps[b], bias)

        def relu_drain(dst):
            def f(m, b, ps, bias):
                nc.scalar.activation(
                    out=dst[:, m, b * 512:(b + 1) * 512], in_=ps,
                    func=mybir.ActivationFunctionType.Relu,
                    bias=bias[:, m:m + 1], scale=1.0,
                )
            return f

        def layer4(wd, bias, src, drain, post_b=None):
            # b-outer so the b=0 half of the output is complete early
            for b in range(NB):
                for m in range(MC_4):
                    ps = mm_ps.tile([128, 512], F32, tag="mm", name="mmL4")
                    for kh in range(0, MC_H, 4):
                        wt = wpool.tile([128, 4, 128], F32R, tag="w")
                        nc.sync.dma_start(out=wt, in_=wd[m, :, kh:kh + 4, :])
                        for kk in range(4):
                            k = kh + kk
                            nc.tensor.matmul(
                                ps, wt[:, kk, :], src(k, b),
                                start=(k == 0), stop=(k == MC_H - 1),
                            )
                    drain(m, b, ps, bias)
                if post_b is not None:
                    post_b(b)

        def mlp(p, last_drain, filler=None, post_l1=None, pre_l4=None,
                l4_post_b=None):
            layer(wdr[f"w1{p}"], wsb[f"b1{p}"],
                  lambda k, b: x1T[:, k, b * 512:(b + 1) * 512],
                  KC_1, MC_H, relu_drain(actA), filler)
            if post_l1 is not None:
                post_l1()
            layer(wdr[f"w2{p}"], wsb[f"b2{p}"],
                  lambda k, b: actA[:, k, b * 512:(b + 1) * 512],
                  MC_H, MC_H, relu_drain(actB), filler)
            layer(wdr[f"w3{p}"], wsb[f"b3{p}"],
                  lambda k, b: actB[:, k, b * 512:(b + 1) * 512],
                  MC_H, MC_H, relu_drain(actA), filler)
            if pre_l4 is not None:
                pre_l4()
            layer4(wdr[f"w4{p}"], wsb[f"b4{p}"],
                  lambda k, b: actA[:, k, b * 512:(b + 1) * 512],
                  last_drain, l4_post_b)

        def in_tr_bt(sb, bt):
            r0 = sb * SB
            xin = io_pool.tile([128, X1], F32R, tag="xin", bufs=3)
            nc.scalar.dma_start(
                out=xin, in_=x_d[r0 + bt * 128: r0 + (bt + 1) * 128, 0:X1].bitcast(F32R))
            for c in range(KC_1):
                tp = tr_ps.tile([128, 128], F32R, tag="trps")
                nc.tensor.transpose(tp, xin[:, c * 128:(c + 1) * 128], idr)
                nc.vector.tensor_copy(
                    out=x1T[:, c, bt * 128:(bt + 1) * 128], in_=tp)

        def in_transpose(sb):
            for bt in range(BT):
                in_tr_bt(sb, bt)

        def s_drain(m, b, ps, bias):
            nc.scalar.activation(
                out=scaleT[:, m, b * 512:(b + 1) * 512], in_=ps,
                func=mybir.ActivationFunctionType.Tanh,
                bias=bias[:, m:m + 1], scale=1.0,
            )

        def ld_exp_b(sb, b):
            # channel-sum of tanh via ones-matmul, then E = exp(scale+og2)
            bc = slice(b * 512, (b + 1) * 512)
            lp = mm_ps.tile([1, 512], F32, tag="mm", name="ldps")
            for c in range(MC_4):
                nc.tensor.matmul(lp, ones, scaleT[:, c, bc],
                                 start=(c == 0), stop=(c == MC_4 - 1))
            lrow = io_pool.tile([1, 512], F32, tag="ldrow", bufs=1)
            nc.scalar.copy(lrow, lp)
            r0 = sb * SB
            nc.scalar.dma_start(
                out=ld_d[(r0 // 512) + b: (r0 // 512) + b + 1, :], in_=lrow)
            for c in range(MC_4):
                nc.scalar.activation(
                    out=scaleT[:, c, bc], in_=scaleT[:, c, bc].bitcast(F32),
                    func=mybir.ActivationFunctionType.Exp,
                    bias=og2[:, c:c + 1], scale=1.0,
                )

        def t_drain(m, b, ps, bias):
            nc.scalar.activation(
                out=trT[:, m, b * 512:(b + 1) * 512], in_=ps,
                func=mybir.ActivationFunctionType.Identity,
                bias=bias[:, m:m + 1], scale=1.0,
            )

        def couple_bt(sb, bt):
            r0 = sb * SB
            rows = slice(r0 + bt * 128, r0 + (bt + 1) * 128)
            onat = io_pool.tile([128, D], F32, tag="onat")
            nc.scalar.dma_start(out=onat[:, 0:X1], in_=x_d[rows, 0:X1])
            nc.scalar.dma_start(out=onat[:, X1:D], in_=x_d[rows, X1:D])

            # first half: x1 * exp(og1) + oc1 (per-channel on free dim)
            nc.vector.tensor_mul(onat[:, 0:X1], onat[:, 0:X1], og1e)
            nc.vector.tensor_add(onat[:, 0:X1], onat[:, 0:X1], oc1)

            # second half: x2 * E + T, transposing E and T back per block
            for c in range(MC_4):
                te = tr_ps.tile([128, 128], F32, tag="trps")
                nc.tensor.transpose(
                    te, scaleT[:, c, bt * 128:(bt + 1) * 128].bitcast(F32), idf)
                tt = tr_ps.tile([128, 128], F32, tag="trps")
                nc.tensor.transpose(
                    tt, trT[:, c, bt * 128:(bt + 1) * 128], idf)
                cs = slice(X1 + c * 128, X1 + (c + 1) * 128)
                nc.vector.tensor_mul(onat[:, cs], onat[:, cs], te)
                nc.vector.tensor_add(onat[:, cs], onat[:, cs], tt)

            nc.scalar.dma_start(out=out_d[rows, :], in_=onat)

        # Deferred-work queues: coupling of sb N-1 is spread through sb N's
        # s-MLP; input transposes of sb N+1 are spread through sb N's t-MLP
        # (emitted only after t-L1, which still reads sb N's x1T).
        pending = []
        fill_state = {"n": 0}

        def filler():
            fill_state["n"] += 1
            if pending and fill_state["n"] % 4 == 0:
                pending.pop(0)()

        def drain_pending():
            while pending:
                pending.pop(0)()

        for sb in range(NSB):
            if sb == 0:
                in_transpose(0)

            # s-MLP: fillers emit couple(sb-1); all must land before s-L4
            # (which overwrites scaleT that couple(sb-1) still reads).
            mlp("s", s_drain, filler, pre_l4=drain_pending,
                l4_post_b=lambda b, sb=sb: ld_exp_b(sb, b))

            def queue_next_in_tr(sb=sb):
                if sb + 1 < NSB:
                    pending.extend(
                        (lambda bt=bt: in_tr_bt(sb + 1, bt)) for bt in range(BT))

            last = sb == NSB - 1

            def couple_b0(sb=sb):
                for bt in range(BT // 2):
                    couple_bt(sb, bt)

            mlp("t", t_drain, filler, post_l1=queue_next_in_tr,
                pre_l4=drain_pending,
                l4_post_b=(lambda b: couple_b0() if b == 0 else None)
                if last else None)

            if last:
                for bt in range(BT // 2, BT):
                    couple_bt(sb, bt)
            else:
                pending.extend(
                    (lambda sb=sb, bt=bt: couple_bt(sb, bt)) for bt in range(BT))
        drain_pending()

    nc.compile()
    return nc


def _prep_inputs(inputs):
    """Host-side folding + layout. Returns the per-core common in_map pieces."""
    i = inputs
    og = np.asarray(i["og"], np.float32)
    oc = np.asarray(i["oc"], np.float32)
    og1e = np.exp(og[:X1])
    oc1 = oc[:X1]
    og2 = og[X1:]
    oc2 = oc[X1:]

    com = {}
    for p, pre in (("s", "s"), ("t", "t")):
        w1, b1, w2, b2, w3, b3, w4, b4 = _fold_mlp(
            *[np.asarray(i[f"{pre}{n}"]) for n in
              ("w1", "b1", "g1", "c1", "w2", "b2", "g2", "c2",
               "w3", "b3", "g3", "c3", "w4", "b4")])
        if p == "t":
            # fold second half of output actnorm into t-MLP's last layer
            e2 = np.exp(og2)
            w4 = (w4 * e2[None, :]).astype(np.float32)
            b4 = (b4 * e2 + oc2).astype(np.float32)
        com[f"w1{p}"] = _wblocks(w1)
        com[f"w2{p}"] = _wblocks(w2)
        com[f"w3{p}"] = _wblocks(w3)
        com[f"w4{p}"] = _wblocks(w4)
        com[f"b1{p}"] = _bblocks(b1)
        com[f"b2{p}"] = _bblocks(b2)
        com[f"b3{p}"] = _bblocks(b3)
        com[f"b4{p}"] = _bblocks(b4)

    com["og1e"] = np.ascontiguousarray(np.broadcast_to(og1e[None, :], (128, X1)))
    com["oc1"] = np.ascontiguousarray(np.broadcast_to(oc1[None, :], (128, X1)))
    com["og2"] = _bblocks(og2)
    com["ones"] = np.ones((128, 1), np.float32)
    com["idf"] = np.eye(128, dtype=np.float32)
    com["idr"] = np.eye(128, dtype=np.float32)
    sum_og = float(np.sum(og))
    return com, sum_og


def _run(inputs, trace=False, tmpdir=None):
    x = np.ascontiguousarray(np.asarray(inputs["x"], np.float32))
    assert x.shape == (B, D)
    com, sum_og = _prep_inputs(inputs)
    nc = _build_program(sum_og)

    in_maps = []
    for c in range(N_CORES):
        m = dict(com)
        m["x"] = np.ascontiguousarray(x[c * BC:(c + 1) * BC])
        in_maps.append(m)

    res = bass_utils.run_bass_kernel_spmd(
        nc, in_maps, core_ids=list(range(N_CORES)), trace=trace, tmpdir=tmpdir)

    out = np.concatenate([r["out"] for r in res.results], axis=0)
    ld = np.concatenate([r["ld"].reshape(-1) for r in res.results], axis=0) + np.float32(sum_og)
    return (out, ld), res


def kernel(**inputs):
    (out, ld), _ = _run(inputs, trace=False)
    return out, ld


# revision 16
# speedup vs baseline: 1.0728x; 1.0141x over previous
"""AffineCoupling forward on 8 TRN2 NeuronCores (Bass/Tile).

Strategy
--------
Data-parallel: batch 16384 is split into 8 shards of 2048 rows; the MLP
weights (~84 MB) are replicated to every core. Inside a core the shard is
processed as 2 superblocks of 1024 rows.

Dataflow: activations live TRANSPOSED in SBUF ([channel-part, batch-free])
so all four layers of both MLPs chain on the TensorEngine without any
transposes; weights are the stationary operand in natural [K, N] layout.
Matmuls run in float32r (TF32-like, ~13-bit mantissa, full PE rate).

Host-side folding: ActNorm (y = x*exp(g)+c) is folded into the previous
layer's weights/bias; the output ActNorm's second half is folded into the
t-MLP's last layer and into the exp() bias of the coupling; the first half
is applied in natural layout with replicated per-channel vectors.
"""

import numpy as np
from contextlib import ExitStack

import concourse.bass as bass
import concourse.tile as tile
from concourse import bacc, mybir, bass_utils

F32 = mybir.dt.float32
F32R = mybir.dt.float32r

B = 16384
D = 1024
X1 = 512
HID = 2048
N_CORES = 8
BC = B // N_CORES          # rows per core = 2048
SB = 1024                  # superblock rows
NSB = BC // SB             # 2
NB = SB // 512             # 2 moving subtiles of 512 per superblock
BT = SB // 128             # 8 b-tiles of 128 per superblock
MC_H = HID // 128          # 16
KC_1 = X1 // 128           # 4
MC_4 = X1 // 128           # 4


def _fold_mlp(w1, b1, g1, c1, w2, b2, g2, c2, w3, b3, g3, c3, w4, b4):
    """Fold the three ActNorms into the linear layers. Returns fp32 arrays."""
    def fold(w, b, g, c):
        e = np.exp(g.astype(np.float32))
        return (w * e[None, :]).astype(np.float32), (b * e + c).astype(np.float32)

    w1f, b1f = fold(w1, b1, g1, c1)
    w2f, b2f = fold(w2, b2, g2, c2)
    w3f, b3f = fold(w3, b3, g3, c3)
    return w1f, b1f, w2f, b2f, w3f, b3f, w4.astype(np.float32), b4.astype(np.float32)


def _wblocks(w):
    """[K, N] -> [MC, 128(kp), KC, 128(mp)] so each DMA line is contiguous."""
    K, N = w.shape
    kc, mc = K // 128, N // 128
    return np.ascontiguousarray(
        w.reshape(kc, 128, mc, 128).transpose(2, 1, 0, 3)
    )


def _bblocks(b):
    """[N] -> [128, MC] (channel chunk on free dim, partition = channel%128)."""
    return np.ascontiguousarray(b.reshape(-1, 128).T)


def _build_program(sum_og: float):
    nc = bacc.Bacc("TRN2", target_bir_lowering=False, debug=False)

    x_d = nc.dram_tensor("x", [BC, D], F32, kind="ExternalInput").ap()
    out_d = nc.dram_tensor("out", [BC, D], F32, kind="ExternalOutput").ap()
    ld_d = nc.dram_tensor("ld", [BC // 512, 512], F32, kind="ExternalOutput").ap()

    wdr = {}
    for p in ("s", "t"):
        wdr[f"w1{p}"] = nc.dram_tensor(f"w1{p}", [MC_H, 128, KC_1, 128], F32R, kind="ExternalInput").ap()
        wdr[f"w2{p}"] = nc.dram_tensor(f"w2{p}", [MC_H, 128, MC_H, 128], F32R, kind="ExternalInput").ap()
        wdr[f"w3{p}"] = nc.dram_tensor(f"w3{p}", [MC_H, 128, MC_H, 128], F32R, kind="ExternalInput").ap()
        wdr[f"w4{p}"] = nc.dram_tensor(f"w4{p}", [MC_4, 128, MC_H, 128], F32R, kind="ExternalInput").ap()
        for i, mc in (("1", MC_H), ("2", MC_H), ("3", MC_H), ("4", MC_4)):
            wdr[f"b{i}{p}"] = nc.dram_tensor(f"b{i}{p}", [128, mc], F32, kind="ExternalInput").ap()

    og1e_d = nc.dram_tensor("og1e", [128, X1], F32, kind="ExternalInput").ap()
    oc1_d = nc.dram_tensor("oc1", [128, X1], F32, kind="ExternalInput").ap()
    og2_d = nc.dram_tensor("og2", [128, MC_4], F32, kind="ExternalInput").ap()
    ones_d = nc.dram_tensor("ones", [128, 1], F32R, kind="ExternalInput").ap()
    idr_d = nc.dram_tensor("idr", [128, 128], F32R, kind="ExternalInput").ap()
    idf_d = nc.dram_tensor("idf", [128, 128], F32, kind="ExternalInput").ap()

    with tile.TileContext(nc) as tc, ExitStack() as ctx:
        consts = ctx.enter_context(tc.tile_pool(name="consts", bufs=1))
        acts = ctx.enter_context(tc.tile_pool(name="acts", bufs=1))
        wpool = ctx.enter_context(tc.tile_pool(name="w", bufs=4))
        io_pool = ctx.enter_context(tc.tile_pool(name="io", bufs=2))
        mm_ps = ctx.enter_context(tc.tile_pool(name="mmps", bufs=4, space="PSUM"))
        tr_ps = ctx.enter_context(tc.tile_pool(name="trps", bufs=4, space="PSUM"))

        # --- identities + PE warmup first (HAM warm before L1; xin DMAs early) ---
        idf = consts.tile([128, 128], F32, tag="idf")
        nc.scalar.dma_start(out=idf, in_=idf_d)
        idr = consts.tile([128, 128], F32R, tag="idr")
        nc.scalar.dma_start(out=idr, in_=idr_d)
        warm_ps = mm_ps.tile([128, 128], F32, tag="mm", name="warmps")
        for wi in range(40):
            nc.tensor.matmul(warm_ps, idr, idr,
                             start=(wi == 0), stop=(wi == 39))
        warm_sb = consts.tile([128, 128], F32, tag="warmsb")
        nc.vector.tensor_copy(out=warm_sb, in_=warm_ps)

        # --- constants ---
        wsb = {}
        for p in ("s", "t"):
            for i, mc in (("1", MC_H), ("2", MC_H), ("3", MC_H), ("4", MC_4)):
                t = consts.tile([128, mc], F32, tag=f"b{i}{p}")
                nc.scalar.dma_start(out=t, in_=wdr[f"b{i}{p}"])
                wsb[f"b{i}{p}"] = t
        og1e = consts.tile([128, X1], F32, tag="og1e")
        nc.scalar.dma_start(out=og1e, in_=og1e_d)
        oc1 = consts.tile([128, X1], F32, tag="oc1")
        nc.scalar.dma_start(out=oc1, in_=oc1_d)
        og2 = consts.tile([128, MC_4], F32, tag="og2")
        nc.scalar.dma_start(out=og2, in_=og2_d)
        ones = consts.tile([128, 1], F32R, tag="ones")
        nc.scalar.dma_start(out=ones, in_=ones_d)

        # --- persistent activation buffers (shared across superblocks) ---
        x1T = acts.tile([128, KC_1, SB], F32R, tag="x1T")
        actA = acts.tile([128, MC_H, SB], F32R, tag="actA")
        actB = acts.tile([128, MC_H, SB], F32R, tag="actB")
        scaleT = acts.tile([128, MC_4, SB], F32R, tag="scaleT")
        trT = acts.tile([128, MC_4, SB], F32, tag="trT")

        def layer(wd, bias, src, kc_n, mc_n, drain, filler=None):
            """One linear layer: out[m-part, b-free] over kc_n k-chunks."""
            for m in range(mc_n):
                if filler is not None:
                    filler()
                ps = [mm_ps.tile([128, 512], F32, tag="mm", name=f"mm{b}")
                      for b in range(NB)]
                for kh in range(0, kc_n, 4):
                    kw = min(4, kc_n - kh)
                    wt = wpool.tile([128, kw, 128], F32R, tag="w")
                    nc.sync.dma_start(out=wt, in_=wd[m, :, kh:kh + kw, :])
                    for kk in range(kw):
                        k = kh + kk
                        for b in range(NB):
                            nc.tensor.matmul(
                                ps[b], wt[:, kk, :], src(k, b),
                                start=(k == 0), stop=(k == kc_n - 1),
                            )
                for b in range(NB):
                    drain(m, b, ps[b], bias)

        def relu_drain(dst):
            def f(m, b, ps, bias):
                nc.vector.tensor_scalar(
                    out=dst[:, m, b * 512:(b + 1) * 512], in0=ps,
                    scalar1=bias[:, m:m + 1], scalar2=0.0,
                    op0=mybir.AluOpType.add, op1=mybir.AluOpType.max,
                )
            return f

        def layer4(wd, bias, src, drain, post_b=None):
            # b-outer so the b=0 half of the output is complete early
            for b in range(NB):
                for m in range(MC_4):
                    ps = mm_ps.tile([128, 512], F32, tag="mm", name="mmL4")
                    for kh in range(0, MC_H, 4):
                        wt = wpool.tile([128, 4, 128], F32R, tag="w")
                        nc.sync.dma_start(out=wt, in_=wd[m, :, kh:kh + 4, :])
                        for kk in range(4):
                            k = kh + kk
                            nc.tensor.matmul(
                                ps, wt[:, kk, :], src(k, b),
                                start=(k == 0), stop=(k == MC_H - 1),
                            )
                    drain(m, b, ps, bias)
                if post_b is not None:
                    post_b(b)

        def mlp(p, last_drain, filler=None, post_l1=None, pre_l4=None,
                l4_post_b=None):
            layer(wdr[f"w1{p}"], wsb[f"b1{p}"],
                  lambda k, b: x1T[:, k, b * 512:(b + 1) * 512],
                  KC_1, MC_H, relu_drain(actA), filler)
            if post_l1 is not None:
                post_l1()
            layer(wdr[f"w2{p}"], wsb[f"b2{p}"],
                  lambda k, b: actA[:, k, b * 512:(b + 1) * 512],
                  MC_H, MC_H, relu_drain(actB), filler)
            layer(wdr[f"w3{p}"], wsb[f"b3{p}"],
                  lambda k, b: actB[:, k, b * 512:(b + 1) * 512],
                  MC_H, MC_H, relu_drain(actA), filler)
            if pre_l4 is not None:
                pre_l4()
            layer4(wdr[f"w4{p}"], wsb[f"b4{p}"],
                  lambda k, b: actA[:, k, b * 512:(b + 1) * 512],
                  last_drain, l4_post_b)

        def in_tr_bt(sb, bt):
            r0 = sb * SB
            xin = io_pool.tile([128, X1], F32R, tag="xin", bufs=3)
            nc.scalar.dma_start(
                out=xin, in_=x_d[r0 + bt * 128: r0 + (bt + 1) * 128, 0:X1].bitcast(F32R))
            for c in range(KC_1):
                tp = tr_ps.tile([128, 128], F32R, tag="trps")
                nc.tensor.transpose(tp, xin[:, c * 128:(c + 1) * 128], idr)
                nc.vector.tensor_copy(
                    out=x1T[:, c, bt * 128:(bt + 1) * 128], in_=tp)

        def in_transpose(sb):
            for bt in range(BT):
                in_tr_bt(sb, bt)

        def s_drain(m, b, ps, bias):
            nc.scalar.activation(
                out=scaleT[:, m, b * 512:(b + 1) * 512], in_=ps,
                func=mybir.ActivationFunctionType.Tanh,
                bias=bias[:, m:m + 1], scale=1.0,
            )

        def ld_exp_b(sb, b):
            # channel-sum of tanh via ones-matmul, then E = exp(scale+og2)
            bc = slice(b * 512, (b + 1) * 512)
            lp = mm_ps.tile([1, 512], F32, tag="mm", name="ldps")
            for c in range(MC_4):
                nc.tensor.matmul(lp, ones, scaleT[:, c, bc],
                                 start=(c == 0), stop=(c == MC_4 - 1))
            lrow = io_pool.tile([1, 512], F32, tag="ldrow", bufs=1)
            nc.scalar.copy(lrow, lp)
            r0 = sb * SB
            nc.scalar.dma_start(
                out=ld_d[(r0 // 512) + b: (r0 // 512) + b + 1, :], in_=lrow)
            for c in range(MC_4):
                nc.scalar.activation(
                    out=scaleT[:, c, bc], in_=scaleT[:, c, bc].bitcast(F32),
                    func=mybir.ActivationFunctionType.Exp,
                    bias=og2[:, c:c + 1], scale=1.0,
                )

        def t_drain(m, b, ps, bias):
            nc.vector.tensor_scalar_add(
                out=trT[:, m, b * 512:(b + 1) * 512], in0=ps,
                scalar1=bias[:, m:m + 1],
            )

        def couple_bt(sb, bt):
            r0 = sb * SB
            rows = slice(r0 + bt * 128, r0 + (bt + 1) * 128)
            onat = io_pool.tile([128, D], F32, tag="onat")
            nc.scalar.dma_start(out=onat[:, 0:X1], in_=x_d[rows, 0:X1])
            nc.scalar.dma_start(out=onat[:, X1:D], in_=x_d[rows, X1:D])

            # first half: x1 * exp(og1) + oc1 (per-channel on free dim)
            nc.vector.tensor_mul(onat[:, 0:X1], onat[:, 0:X1], og1e)
            nc.vector.tensor_add(onat[:, 0:X1], onat[:, 0:X1], oc1)

            # second half: x2 * E + T, transposing E and T back per block
            for c in range(MC_4):
                te = tr_ps.tile([128, 128], F32, tag="trps")
                nc.tensor.transpose(
                    te, scaleT[:, c, bt * 128:(bt + 1) * 128].bitcast(F32), idf)
                tt = tr_ps.tile([128, 128], F32, tag="trps")
                nc.tensor.transpose(
                    tt, trT[:, c, bt * 128:(bt + 1) * 128], idf)
                cs = slice(X1 + c * 128, X1 + (c + 1) * 128)
                nc.vector.tensor_mul(onat[:, cs], onat[:, cs], te)
                nc.vector.tensor_add(onat[:, cs], onat[:, cs], tt)

            nc.scalar.dma_start(out=out_d[rows, :], in_=onat)

        # Deferred-work queues: coupling of sb N-1 is spread through sb N's
        # s-MLP; input transposes of sb N+1 are spread through sb N's t-MLP
        # (emitted only after t-L1, which still reads sb N's x1T).
        pending = []
        fill_state = {"n": 0}

        def filler():
            fill_state["n"] += 1
            if pending and fill_state["n"] % 4 == 0:
                pending.pop(0)()

        def drain_pending():
            while pending:
                pending.pop(0)()

        for sb in range(NSB):
            if sb == 0:
                in_transpose(0)

            # s-MLP: fillers emit couple(sb-1); all must land before s-L4
            # (which overwrites scaleT that couple(sb-1) still reads).
            mlp("s", s_drain, filler, pre_l4=drain_pending,
                l4_post_b=lambda b, sb=sb: ld_exp_b(sb, b))

            def queue_next_in_tr(sb=sb):
                if sb + 1 < NSB:
                    pending.extend(
                        (lambda bt=bt: in_tr_bt(sb + 1, bt)) for bt in range(BT))

            last = sb == NSB - 1

            def couple_b0(sb=sb):
                for bt in range(BT // 2):
                    couple_bt(sb, bt)

            mlp("t", t_drain, filler, post_l1=queue_next_in_tr,
                pre_l4=drain_pending,
                l4_post_b=(lambda b: couple_b0() if b == 0 else None)
                if last else None)

            if last:
                for bt in range(BT // 2, BT):
                    couple_bt(sb, bt)
            else:
                pending.extend(
                    (lambda sb=sb, bt=bt: couple_bt(sb, bt)) for bt in range(BT))
        drain_pending()

    nc.compile()
    return nc


def _prep_inputs(inputs):
    """Host-side folding + layout. Returns the per-core common in_map pieces."""
    i = inputs
    og = np.asarray(i["og"], np.float32)
    oc = np.asarray(i["oc"], np.float32)
    og1e = np.exp(og[:X1])
    oc1 = oc[:X1]
    og2 = og[X1:]
    oc2 = oc[X1:]

    com = {}
    for p, pre in (("s", "s"), ("t", "t")):
        w1, b1, w2, b2, w3, b3, w4, b4 = _fold_mlp(
            *[np.asarray(i[f"{pre}{n}"]) for n in
              ("w1", "b1", "g1", "c1", "w2", "b2", "g2", "c2",
               "w3", "b3", "g3", "c3", "w4", "b4")])
        if p == "t":
            # fold second half of output actnorm into t-MLP's last layer
            e2 = np.exp(og2)
            w4 = (w4 * e2[None, :]).astype(np.float32)
            b4 = (b4 * e2 + oc2).astype(np.float32)
        com[f"w1{p}"] = _wblocks(w1)
        com[f"w2{p}"] = _wblocks(w2)
        com[f"w3{p}"] = _wblocks(w3)
        com[f"w4{p}"] = _wblocks(w4)
        com[f"b1{p}"] = _bblocks(b1)
        com[f"b2{p}"] = _bblocks(b2)
        com[f"b3{p}"] = _bblocks(b3)
        com[f"b4{p}"] = _bblocks(b4)

    com["og1e"] = np.ascontiguousarray(np.broadcast_to(og1e[None, :], (128, X1)))
    com["oc1"] = np.ascontiguousarray(np.broadcast_to(oc1[None, :], (128, X1)))
    com["og2"] = _bblocks(og2)
    com["ones"] = np.ones((128, 1), np.float32)
    com["idf"] = np.eye(128, dtype=np.float32)
    com["idr"] = np.eye(128, dtype=np.float32)
    sum_og = float(np.sum(og))
    return com, sum_og


def _run(inputs, trace=False, tmpdir=None):
    x = np.ascontiguousarray(np.asarray(inputs["x"], np.float32))
    assert x.shape == (B, D)
    com, sum_og = _prep_inputs(inputs)
    nc = _build_program(sum_og)

    in_maps = []
    for c in range(N_CORES):
        m = dict(com)
        m["x"] = np.ascontiguousarray(x[c * BC:(c + 1) * BC])
        in_maps.append(m)

    res = bass_utils.run_bass_kernel_spmd(
        nc, in_maps, core_ids=list(range(N_CORES)), trace=trace, tmpdir=tmpdir)

    out = np.concatenate([r["out"] for r in res.results], axis=0)
    ld = np.concatenate([r["ld"].reshape(-1) for r in res.results], axis=0) + np.float32(sum_og)
    return (out, ld), res


def kernel(**inputs):
    (out, ld), _ = _run(inputs, trace=False)
    return out, ld
